# revision 50
# baseline (speedup 1.0000x reference)
"""DeepseekV3 MoE (T=1024, D=2048, E=32, grouped top-4 routing, I=1408,
shared expert 2816) on 8 trn2 NeuronCores via Bass/Tile.

Expert-parallel sparse dispatch; see build_program for the device-side
structure. Host computes the gate forward once (numpy) ONLY to size
per-expert capacities and balance the expert->(core,slot) assignment;
all model math runs on device.
"""

import numpy as np
import ml_dtypes
from einops import rearrange

import concourse.bass as bass
import concourse.bacc as bacc
import concourse.mybir as mybir
from concourse.tile import TileContext
from concourse.bass_utils import run_bass_kernel_spmd

F32 = mybir.dt.float32
BF16 = mybir.dt.bfloat16
I32 = mybir.dt.int32

T, D, E, I = 1024, 2048, 32, 1408
G, TOPK_GROUP, TOP_K = 8, 4, 4
SHARED_I = 2816
ROUTED_SCALING = 2.5
NC = 8
P = 128
NT = T // P            # 8 token tiles
ND = D // P            # 16 d chunks
NI = I // P            # 11 i tiles
SI = SHARED_I // NC    # 352 per-core shard of the shared intermediate
SIP = 384              # padded to 3 tiles of 128
NSI = SIP // P         # 3
BIG = 60000.0          # offset for non-selected rows; > bounds -> skipped
COUNT_GUARD = 4
HD = D // 2            # 1024: column half for split reduce-scatter

_program_cache = {}


def _routing_counts(x, gate_w, gate_b):
    logits = x.astype(np.float32) @ gate_w.T.astype(np.float32) + gate_b
    scores = 1.0 / (1.0 + np.exp(-logits))
    s4c = scores + gate_b
    grp = s4c.reshape(T, G, E // G)
    m1 = grp.max(-1)
    m2 = np.where(grp == m1[:, :, None], -np.inf, grp).max(-1)
    gs = m1 + m2
    th = np.sort(gs, 1)[:, G - TOPK_GROUP]
    smask = np.repeat(gs >= th[:, None], E // G, axis=1)
    tmp = np.where(smask, s4c, 0.0)
    et = np.sort(tmp, 1)[:, E - TOP_K]
    return (tmp >= et[:, None]).sum(0)


def _assignment(counts):
    blocks = np.maximum(1, np.ceil((counts + COUNT_GUARD) / P)).astype(int)
    order = sorted(range(E), key=lambda e: (-blocks[e], -counts[e]))
    prof = tuple(int(blocks[order[8 * k]]) for k in range(4))
    assign = np.zeros((NC, 4), dtype=int)
    for k in range(4):
        for c in range(NC):
            assign[c, k] = order[8 * k + c]
    return assign, prof


def build_program(prof):
    nc = bacc.Bacc("TRN2", num_devices=NC)
    CK = [P * b for b in prof]
    CKMAX = max(CK)

    xTf = nc.dram_tensor("xTf", [ND, P, T], F32, kind="ExternalInput")
    xbf = nc.dram_tensor("xbf", [T + CKMAX, D], BF16, kind="ExternalInput")
    gwT = nc.dram_tensor("gwT", [P, ND, E], F32, kind="ExternalInput")
    gbias = nc.dram_tensor("gbias", [1, E], F32, kind="ExternalInput")
    ones1 = nc.dram_tensor("ones1", [1, P], F32, kind="ExternalInput")
    ut128 = nc.dram_tensor("ut128", [P, P], F32, kind="ExternalInput")
    wcum = nc.dram_tensor("wcum", [P, NT * NT], F32, kind="ExternalInput")
    ejall = nc.dram_tensor("ejall", [NT, NT * P], F32, kind="ExternalInput")
    onehot = nc.dram_tensor("onehot", [P, 4, E], F32, kind="ExternalInput")
    iota128 = nc.dram_tensor("iota128", [P, 1], F32, kind="ExternalInput")
    initmeta = nc.dram_tensor("initmeta", [CKMAX, 2], F32, kind="ExternalInput")
    wgu = [nc.dram_tensor(f"wgu{k}", [NI, P, ND, P], BF16, kind="ExternalInput")
           for k in range(4)]
    wuu = [nc.dram_tensor(f"wuu{k}", [NI, P, ND, P], BF16, kind="ExternalInput")
           for k in range(4)]
    wdd = [nc.dram_tensor(f"wdd{k}", [4, P, NI * 512], BF16,
                          kind="ExternalInput") for k in range(4)]
    swg = nc.dram_tensor("swg", [NSI, P, ND, P], BF16, kind="ExternalInput")
    swu = nc.dram_tensor("swu", [NSI, P, ND, P], BF16, kind="ExternalInput")
    swd = nc.dram_tensor("swd", [4, P, NSI * 512], BF16, kind="ExternalInput")

    ysh = nc.dram_tensor("ysh", [P, D], BF16, kind="ExternalOutput")

    AF = mybir.ActivationFunctionType
    ALU = mybir.AluOpType

    with TileContext(nc) as tc:
        with tc.tile_pool(name="dram", bufs=1, space="DRAM") as dpool, \
             tc.tile_pool(name="const", bufs=1) as cpool, \
             tc.tile_pool(name="small", bufs=3) as spool, \
             tc.tile_pool(name="meta", bufs=10) as mpool, \
             tc.tile_pool(name="wbpool", bufs=2) as wbpool:

            # per-column-half partial sums (routed scatter-adds RMW into
            # these; shared expert initializes them)
            partial = [dpool.tile([T + CKMAX, HD], BF16, tag=f"partial{h}",
                                  name=f"partial{h}") for h in range(2)]
            rsout = [dpool.tile([P, HD], BF16, tag=f"rsout{h}",
                                name=f"rsout{h}") for h in range(2)]
            glmeta = [dpool.tile([CK[k], 2], F32, tag=f"glmeta{k}",
                                 name=f"glmeta{k}") for k in range(4)]

            ut_sb = cpool.tile([P, P], F32, tag="ut")
            wcum_sb = cpool.tile([P, NT * NT], F32, tag="wcum")
            ej_sb = cpool.tile([NT, NT * P], F32, tag="ej")
            oh_sb = cpool.tile([P, 4, E], F32, tag="oh")
            ones1_sb = cpool.tile([1, P], F32, tag="ones1")
            iota_sb = cpool.tile([P, 1], F32, tag="iota")
            gb_sb = cpool.tile([1, E], F32, tag="gb")
            gw_all = cpool.tile([P, ND, E], F32, tag="gwall")
            bias_bc = cpool.tile([P, E], F32, tag="biasbc")
            routew = cpool.tile([P, NT, E], F32, tag="routew")
            poff_all = cpool.tile([P, NT, E], F32, tag="poffall")
            base8 = cpool.tile([NT, E], F32, tag="base8")
            a_sh = cpool.tile([P, NSI, T], BF16, tag="ash")

            # gate-critical loads first (everything else after the gate
            # matmul emission so its triggers don't delay the xf stream)
            nc.sync.dma_start(out=gw_all[:], in_=gwT[:])
            nc.sync.dma_start(out=ones1_sb[:], in_=ones1[:])
            nc.sync.dma_start(out=gb_sb[:], in_=gbias[:])

            # ===== phase 1: gate + routing + shared up/gate + positions ====
            # xTbf (32KB/partition) + shared weights live only in this scope
            xtb_ctx = tc.tile_pool(name="xtbp", bufs=1)
            xtbp = xtb_ctx.__enter__()
            xTbf = xtbp.tile([P, ND, T], BF16, tag="xTbf")

            # gate (f32), computed as logits^T [E, T]: gw stationary, x
            # moving with 512-wide free dim, then an exact 3-way bf16
            # split + XBAR-DMA transpose back to [T, E] layout
            with tc.tile_pool(name="pgate", bufs=2, space="PSUM") as pg, \
                 tc.tile_pool(name="xtfs", bufs=2) as xpool, \
                 tc.tile_pool(name="gtp", bufs=1) as gtp:
                psb = pg.tile([P, E], F32, tag="psb")
                nc.tensor.matmul(psb[:], ones1_sb[:], gb_sb[:],
                                 start=True, stop=True)
                nc.vector.tensor_copy(out=bias_bc[:], in_=psb[:])
                st = [pg.tile([E, 512], F32, tag="stps", name=f"st{h}")
                      for h in range(2)]
                for dc in range(ND):
                    xf = xpool.tile([P, T], F32, tag="xtf")
                    nc.sync.dma_start(out=xf[:], in_=xTf[dc])
                    nc.scalar.activation(xTbf[:, dc, :], xf[:], AF.Copy)
                    for h in range(2):
                        nc.tensor.matmul(st[h][:], gw_all[:, dc, :],
                                         xf[:, h * 512:(h + 1) * 512],
                                         start=(dc == 0), stop=(dc == ND - 1))

                # non-critical const loads + shared-weight prefetch (their
                # sync-queue triggers now sit behind the xf stream)
                nc.sync.dma_start(out=ut_sb[:], in_=ut128[:])
                nc.sync.dma_start(out=wcum_sb[:], in_=wcum[:])
                nc.sync.dma_start(out=ej_sb[:], in_=ejall[:])
                nc.sync.dma_start(out=oh_sb[:], in_=onehot[:])
                nc.sync.dma_start(out=iota_sb[:], in_=iota128[:])
                for k in range(4):
                    nc.scalar.dma_start(out=glmeta[k][:],
                                        in_=initmeta[:CK[k], :])
                wsh_g, wsh_u = [], []
                for it in range(NSI):
                    wg_sh = xtbp.tile([P, ND, P], BF16, tag=f"swgt{it}",
                                      name=f"swgt{it}")
                    nc.sync.dma_start(out=wg_sh[:], in_=swg[it])
                    wsh_g.append(wg_sh)
                    wu_sh = xtbp.tile([P, ND, P], BF16, tag=f"swut{it}",
                                      name=f"swut{it}")
                    nc.sync.dma_start(out=wu_sh[:], in_=swu[it])
                    wsh_u.append(wu_sh)

                st_sb = gtp.tile([E, T], F32, tag="stsb")
                for h in range(2):
                    nc.vector.tensor_copy(out=st_sb[:, h * 512:(h + 1) * 512],
                                          in_=st[h][:])
                hi = gtp.tile([E, T], BF16, tag="hi")
                nc.vector.tensor_copy(out=hi[:], in_=st_sb[:])
                r1 = gtp.tile([E, T], F32, tag="r1")
                nc.vector.tensor_tensor(out=r1[:], in0=st_sb[:], in1=hi[:],
                                        op=ALU.subtract)
                mid = gtp.tile([E, T], BF16, tag="mid")
                nc.vector.tensor_copy(out=mid[:], in_=r1[:])
                r2 = gtp.tile([E, T], F32, tag="r2")
                nc.vector.tensor_tensor(out=r2[:], in0=r1[:], in1=mid[:],
                                        op=ALU.subtract)
                lo = gtp.tile([E, T], BF16, tag="lo")
                nc.vector.tensor_copy(out=lo[:], in_=r2[:])
                hiT = gtp.tile([P, NT, E], BF16, tag="hiT")
                midT = gtp.tile([P, NT, E], BF16, tag="midT")
                loT = gtp.tile([P, NT, E], BF16, tag="loT")
                nc.scalar.dma_start_transpose(out=hiT[:], in_=hi[:])
                nc.scalar.dma_start_transpose(out=midT[:], in_=mid[:])
                nc.scalar.dma_start_transpose(out=loT[:], in_=lo[:])
                bias_wide = bias_bc[:, None, :].to_broadcast([P, NT, E])
                logitsT = gtp.tile([P, NT, E], F32, tag="logitsT")
                nc.vector.tensor_tensor(out=logitsT[:], in0=hiT[:],
                                        in1=midT[:], op=ALU.add)
                nc.vector.tensor_add(logitsT[:], logitsT[:], loT[:])
                nc.vector.tensor_tensor(out=logitsT[:], in0=logitsT[:],
                                        in1=bias_wide, op=ALU.add)
                scores_all = gtp.tile([P, NT, E], F32, tag="scoresall")
                nc.scalar.activation(scores_all[:], logitsT[:], AF.Sigmoid)
                s4c_all = gtp.tile([P, NT, E], F32, tag="s4call")
                nc.vector.tensor_tensor(out=s4c_all[:], in0=scores_all[:],
                                        in1=bias_wide, op=ALU.add)

                # ============ routing: wide ops over all 8 token tiles ====
                R = E // G
                s4g = s4c_all[:].rearrange("p j (g r) -> p (j g) r", r=R)
                m1a = gtp.tile([P, NT * G, 1], F32, tag="m1a")
                nc.vector.reduce_max(out=m1a[:], in_=s4g,
                                     axis=mybir.AxisListType.X)
                mska = gtp.tile([P, NT, E], F32, tag="mska")
                mskg = mska[:].rearrange("p j (g r) -> p (j g) r", r=R)
                nc.vector.tensor_tensor(out=mskg, in0=s4g,
                                        in1=m1a[:].to_broadcast(
                                            [P, NT * G, R]),
                                        op=ALU.is_equal)
                nc.vector.tensor_scalar(out=mska[:], in0=mska[:], scalar1=1e9,
                                        scalar2=None, op0=ALU.mult)
                nc.vector.tensor_sub(mska[:], s4c_all[:], mska[:])
                m2a = gtp.tile([P, NT * G, 1], F32, tag="m2a")
                nc.vector.reduce_max(out=m2a[:], in_=mskg,
                                     axis=mybir.AxisListType.X)
                gscall = gtp.tile([P, NT, G], F32, tag="gscall")
                nc.vector.tensor_tensor(
                    out=gscall[:].rearrange("p j g -> p (j g)"),
                    in0=m1a[:, :, 0], in1=m2a[:, :, 0], op=ALU.add)
                nc.vector.tensor_scalar(out=gscall[:], in0=gscall[:],
                                        scalar1=4.0, scalar2=None,
                                        op0=ALU.add)
                gzall = gtp.tile([P, NT, G], F32, tag="gzall")
                for j in range(NT):
                    mx8 = spool.tile([P, 8], F32, tag="mx8")
                    nc.vector.max(out=mx8[:], in_=gscall[:, j, :])
                    nc.vector.memset(mx8[:, TOPK_GROUP:], 0.0)
                    nc.vector.match_replace(out=gzall[:, j, :],
                                            in_to_replace=mx8[:],
                                            in_values=gscall[:, j, :],
                                            imm_value=0.0)
                gmka = gtp.tile([P, NT, G], F32, tag="gmka")
                nc.vector.tensor_tensor(out=gmka[:], in0=gscall[:],
                                        in1=gzall[:], op=ALU.subtract)
                nc.vector.tensor_scalar(out=gmka[:], in0=gmka[:], scalar1=0.0,
                                        scalar2=None, op0=ALU.is_gt)
                stmpa = gtp.tile([P, NT, E], F32, tag="stmpa")
                nc.vector.tensor_scalar(out=stmpa[:], in0=s4c_all[:],
                                        scalar1=4.0, scalar2=None,
                                        op0=ALU.add)
                nc.vector.tensor_tensor(
                    out=stmpa[:].rearrange("p j (g r) -> p (j g) r", r=R),
                    in0=stmpa[:].rearrange("p j (g r) -> p (j g) r", r=R),
                    in1=gmka[:].rearrange("p j g -> p (j g)")[:, :, None]
                        .to_broadcast([P, NT * G, R]),
                    op=ALU.mult)
                ezall = gtp.tile([P, NT, E], F32, tag="ezall")
                for j in range(NT):
                    ex8 = spool.tile([P, 8], F32, tag="ex8")
                    nc.vector.max(out=ex8[:], in_=stmpa[:, j, :])
                    nc.vector.memset(ex8[:, TOP_K:], 0.0)
                    nc.vector.match_replace(out=ezall[:, j, :],
                                            in_to_replace=ex8[:],
                                            in_values=stmpa[:, j, :],
                                            imm_value=0.0)
                nc.vector.tensor_sub(stmpa[:], stmpa[:], ezall[:])
                nc.vector.tensor_scalar(out=poff_all[:], in0=stmpa[:],
                                        scalar1=0.0, scalar2=None,
                                        op0=ALU.is_gt)
                twa = gtp.tile([P, NT, E], F32, tag="twa")
                nc.vector.tensor_tensor(out=twa[:], in0=scores_all[:],
                                        in1=poff_all[:], op=ALU.mult)
                rsa = gtp.tile([P, NT, 1], F32, tag="rsa")
                nc.vector.reduce_sum(out=rsa[:], in_=twa[:],
                                     axis=mybir.AxisListType.X)
                nc.vector.reciprocal(out=rsa[:], in_=rsa[:])
                nc.vector.tensor_scalar(out=rsa[:], in0=rsa[:],
                                        scalar1=ROUTED_SCALING,
                                        scalar2=None, op0=ALU.mult)
                nc.vector.tensor_tensor(out=routew[:], in0=twa[:],
                                        in1=rsa[:].to_broadcast([P, NT, E]),
                                        op=ALU.mult)

            # ============ shared expert up/gate (dense over all T) ========
            with tc.tile_pool(name="psh", bufs=3, space="PSUM") as psh:
                for th in range(2):
                    tsl = slice(th * 512, (th + 1) * 512)
                    for it in range(NSI):
                        h1 = psh.tile([P, 512], F32, tag="psh")
                        h2 = psh.tile([P, 512], F32, tag="psh")
                        for dc in range(ND):
                            nc.tensor.matmul(h1[:], wsh_g[it][:, dc, :],
                                             xTbf[:, dc, tsl],
                                             start=(dc == 0),
                                             stop=(dc == ND - 1))
                        for dc in range(ND):
                            nc.tensor.matmul(h2[:], wsh_u[it][:, dc, :],
                                             xTbf[:, dc, tsl],
                                             start=(dc == 0),
                                             stop=(dc == ND - 1))
                        sl = spool.tile([P, 512], F32, tag="silu")
                        nc.scalar.activation(sl[:], h1[:], AF.Silu)
                        nc.vector.tensor_tensor(out=a_sh[:, it, tsl],
                                                in0=sl[:], in1=h2[:],
                                                op=ALU.mult)

            # ============ slot positions (cumsum via matmuls) ============
            with tc.tile_pool(name="ppos", bufs=3, space="PSUM") as pp:
                pb8 = pp.tile([NT, E], F32, tag="pbase")
                for jp in range(NT):
                    nc.tensor.matmul(pb8[:], wcum_sb[:, jp * NT:(jp + 1) * NT],
                                     poff_all[:, jp, :],
                                     start=(jp == 0), stop=(jp == NT - 1))
                nc.vector.tensor_copy(out=base8[:], in_=pb8[:])
                for j in range(NT):
                    ppos = pp.tile([P, E], F32, tag="ppos")
                    nc.tensor.matmul(ppos[:], ut_sb[:], poff_all[:, j, :],
                                     start=True, stop=False)
                    nc.tensor.matmul(ppos[:], ej_sb[:, j * P:(j + 1) * P],
                                     base8[:], start=False, stop=True)
                    t1 = spool.tile([P, E], F32, tag="t1")
                    nc.vector.tensor_scalar(out=t1[:], in0=poff_all[:, j, :],
                                            scalar1=-BIG, scalar2=BIG,
                                            op0=ALU.mult, op1=ALU.add)
                    p2 = spool.tile([P, E], F32, tag="p2")
                    nc.vector.tensor_scalar(out=p2[:], in0=ppos[:], scalar1=1.0,
                                            scalar2=None, op0=ALU.subtract)
                    nc.vector.tensor_mul(p2[:], p2[:], poff_all[:, j, :])
                    nc.vector.tensor_add(poff_all[:, j, :], p2[:], t1[:])

            # close phase-1 scope: frees xTbf's 32KB/partition
            xtb_ctx.__exit__(None, None, None)

            # ============ dispatch: inversion scatters ============
            # all 4 expert-slots' (position, token, weight) computed in one
            # set of wide ops per token tile; scatters emitted per-k later
            def scatter_meta_j(j):
                sel4 = mpool.tile([P, 4, E], F32, tag="sel4", bufs=8,
                                  name=f"sel4_{j}")
                nc.vector.tensor_tensor(
                    out=sel4[:], in0=poff_all[:, j, None, :].to_broadcast(
                        [P, 4, E]), in1=oh_sb[:], op=ALU.mult)
                offk4 = mpool.tile([P, 4, 1], F32, tag="offk4", bufs=8,
                                   name=f"offk4_{j}")
                nc.vector.reduce_sum(out=offk4[:], in_=sel4[:],
                                     axis=mybir.AxisListType.X)
                offi4 = mpool.tile([P, 4], I32, tag="offi4", bufs=8,
                                   name=f"offi4_{j}")
                nc.vector.tensor_copy(out=offi4[:], in_=offk4[:, :, 0])
                meta4 = mpool.tile([P, 4, 2], F32, tag="meta4", bufs=8,
                                   name=f"meta4_{j}")
                nc.vector.tensor_scalar(
                    out=meta4[:, :, 0], in0=iota_sb[:, :1].to_broadcast(
                        [P, 4]), scalar1=float(j * P), scalar2=None,
                    op0=ALU.add)
                wsel4 = mpool.tile([P, 4, E], F32, tag="wsel4", bufs=8,
                                   name=f"wsel4_{j}")
                nc.vector.tensor_tensor(
                    out=wsel4[:], in0=routew[:, j, None, :].to_broadcast(
                        [P, 4, E]), in1=oh_sb[:], op=ALU.mult)
                nc.vector.reduce_sum(out=meta4[:, :, 1:2], in_=wsel4[:],
                                     axis=mybir.AxisListType.X)
                return meta4, offi4

            # ====== shared expert + routed experts ======
            with tc.tile_pool(name="ph", bufs=4, space="PSUM") as ph, \
                 tc.tile_pool(name="py", bufs=4, space="PSUM") as py, \
                 tc.tile_pool(name="wpool", bufs=3) as wpool, \
                 tc.tile_pool(name="wdpool", bufs=3) as wdpool, \
                 tc.tile_pool(name="apool", bufs=2) as apool, \
                 tc.tile_pool(name="gpool", bufs=1) as gpool, \
                 tc.tile_pool(name="ypool", bufs=6) as ypool:

                # dispatch chain: meta + x gather + XBAR-DMA transpose;
                # ms tiles stay alive: ms[:, 1] scales the down-proj output
                def dispatch_gather(k):
                    ck = CK[k]
                    nblk = prof[k]
                    toki, mss = [], []
                    for b in range(nblk):
                        ms = wbpool.tile([P, 2], F32, tag="metald", bufs=7,
                                         name=f"metald{k}_{b}")
                        nc.gpsimd.dma_start(out=ms[:],
                                            in_=glmeta[k][b * P:(b + 1) * P, :])
                        ti = wbpool.tile([P, 1], I32, tag="toki", bufs=7,
                                         name=f"toki{k}_{b}")
                        nc.vector.tensor_copy(out=ti[:], in_=ms[:, 0:1])
                        toki.append(ti)
                        mss.append(ms)
                    xgT = gpool.tile([P, ND, ck], BF16, tag=f"xgT{k}",
                                     name=f"xgT{k}")
                    for b in range(nblk):
                        xg = gpool.tile([P, D], BF16, tag="xg", bufs=3,
                                        name=f"xg{k}_{b}")
                        nc.gpsimd.indirect_dma_start(
                            out=xg[:], out_offset=None,
                            in_=xbf[:],
                            in_offset=bass.IndirectOffsetOnAxis(
                                ap=toki[b][:, :1], axis=0),
                        )
                        nc.scalar.dma_start_transpose(
                            out=xgT[:, :, b * P:(b + 1) * P], in_=xg[:])
                    return toki, mss, xgT

                # preload shared-down weights before dispatch clogs queues
                swd_sb = []
                for db in range(4):
                    w = wdpool.tile([P, NSI * 512], BF16, tag="swd", bufs=4,
                                    name=f"swd{db}")
                    nc.sync.dma_start(out=w[:], in_=swd[db])
                    swd_sb.append(w)

                metas = [scatter_meta_j(j) for j in range(NT)]
                disp = {}
                for k in range(4):
                    for j in range(NT):
                        meta4, offi4 = metas[j]
                        nc.gpsimd.indirect_dma_start(
                            out=glmeta[k][:],
                            out_offset=bass.IndirectOffsetOnAxis(
                                ap=offi4[:, k:k + 1], axis=0),
                            in_=meta4[:, k, :],
                            in_offset=None,
                            bounds_check=CK[k] - 1,
                            oob_is_err=False,
                        )
                    disp[k] = dispatch_gather(k)

                # shared down -> init partial halves
                for db in range(4):
                    w = swd_sb[db]
                    hh = db // 2
                    dsl = slice((db % 2) * 512, (db % 2) * 512 + 512)
                    for ts in range(NT):
                        pys = py.tile([P, 512], F32, tag="py")
                        for ic in range(NSI):
                            nc.tensor.matmul(
                                pys[:],
                                a_sh[:, ic, ts * P:(ts + 1) * P],
                                w[:, ic * 512:(ic + 1) * 512],
                                start=(ic == 0), stop=(ic == NSI - 1))
                        outt = spool.tile([P, 512], BF16, tag="sout")
                        nc.vector.tensor_copy(out=outt[:], in_=pys[:])
                        nc.sync.dma_start(
                            out=partial[hh][ts * P:(ts + 1) * P, dsl],
                            in_=outt[:])

                # routed experts
                for k in range(4):
                    ck = CK[k]
                    nblk = prof[k]
                    toki, mss, xgT = disp.pop(k)
                    # up/gate (unscaled; routing weight applied on the
                    # down-proj output rows)
                    a_e = apool.tile([P, NI, ck], BF16, tag="ae",
                                     name=f"ae{k}")
                    for itp in range(6):
                        nit = 2 if itp < 5 else 1
                        wg_sb = wpool.tile([P, 2, ND, P], BF16, tag="wgu")
                        nc.sync.dma_start(
                            out=wg_sb[:, :nit],
                            in_=wgu[k][2 * itp:2 * itp + nit].rearrange(
                                "j p c i -> p j c i"))
                        wu_sb = wpool.tile([P, 2, ND, P], BF16, tag="wuu")
                        nc.sync.dma_start(
                            out=wu_sb[:, :nit],
                            in_=wuu[k][2 * itp:2 * itp + nit].rearrange(
                                "j p c i -> p j c i"))
                        for jt in range(nit):
                            it = 2 * itp + jt
                            h1 = ph.tile([P, 512], F32, tag="ph")
                            h2 = ph.tile([P, 512], F32, tag="ph")
                            for dc in range(ND):
                                nc.tensor.matmul(h1[:, :ck], wg_sb[:, jt, dc, :],
                                                 xgT[:, dc, :],
                                                 start=(dc == 0),
                                                 stop=(dc == ND - 1))
                            for dc in range(ND):
                                nc.tensor.matmul(h2[:, :ck], wu_sb[:, jt, dc, :],
                                                 xgT[:, dc, :],
                                                 start=(dc == 0),
                                                 stop=(dc == ND - 1))
                            sl = spool.tile([P, 512], F32, tag="silu")
                            nc.scalar.activation(sl[:, :ck], h1[:, :ck], AF.Silu)
                            nc.vector.tensor_tensor(out=a_e[:, it, :],
                                                    in0=sl[:, :ck],
                                                    in1=h2[:, :ck],
                                                    op=ALU.mult)
                    # down projection -> weighted y, staged per column half
                    # then scatter-added as soon as each half is complete
                    ysts = [[ypool.tile([P, HD], BF16, tag="yst",
                                        name=f"yst{k}_{b}_{h}")
                             for h in range(2)] for b in range(nblk)]
                    for db in range(4):
                        hh = db // 2
                        dsl = slice((db % 2) * 512, (db % 2) * 512 + 512)
                        pys = [py.tile([P, 512], F32, tag="py", name=f"pys{b}")
                               for b in range(nblk)]
                        wd_sb = wdpool.tile([P, NI * 512], BF16, tag="wd",
                                            name=f"wd{k}_{db}")
                        nc.sync.dma_start(out=wd_sb[:], in_=wdd[k][db])
                        for ic in range(NI):
                            for b in range(nblk):
                                nc.tensor.matmul(
                                    pys[b][:],
                                    a_e[:, ic, b * P:(b + 1) * P],
                                    wd_sb[:, ic * 512:(ic + 1) * 512],
                                    start=(ic == 0), stop=(ic == NI - 1))
                        for b in range(nblk):
                            nc.vector.tensor_scalar_mul(
                                out=ysts[b][hh][:, dsl], in0=pys[b][:],
                                scalar1=mss[b][:, 1:2])
                            if db % 2 == 1:
                                nc.gpsimd.indirect_dma_start(
                                    out=partial[hh][:],
                                    out_offset=bass.IndirectOffsetOnAxis(
                                        ap=toki[b][:, :1], axis=0),
                                    in_=ysts[b][hh][:],
                                    in_offset=None,
                                    compute_op=ALU.add,
                                )
                        if k == 3 and db == 1:
                            # all half-0 scatter-adds are emitted; start
                            # the first reduce-scatter while half-1 is
                            # still being computed
                            nc.gpsimd.collective_compute(
                                "ReduceScatter",
                                mybir.AluOpType.add,
                                ins=[partial[0][0:T, :]],
                                outs=[rsout[0][:]],
                                replica_groups=[list(range(NC))],
                            )
                            nc.scalar.dma_start(out=ysh[:, 0:HD],
                                                in_=rsout[0][:])

                nc.gpsimd.collective_compute(
                    "ReduceScatter",
                    mybir.AluOpType.add,
                    ins=[partial[1][0:T, :]],
                    outs=[rsout[1][:]],
                    replica_groups=[list(range(NC))],
                )
                nc.scalar.dma_start(out=ysh[:, HD:D], in_=rsout[1][:])

    nc.finalize()
    return nc


def _bf16(a):
    return np.ascontiguousarray(a.astype(ml_dtypes.bfloat16))


def _wd_flat(wd, ni):
    """[I', D] -> [4 dblocks, 128, ni*512]: line (db, p) is the concat over
    ic of wd[ic*128+p, db*512:(db+1)*512] (long contiguous DMA lines)."""
    ipad = ni * P
    wdp = np.zeros((ipad, D), np.float32)
    wdp[:wd.shape[0]] = wd
    a = wdp.reshape(ni, P, 4, 512)
    out = np.transpose(a, (2, 1, 0, 3)).reshape(4, P, ni * 512)
    return _bf16(out)


def _prep_inputs(x, gate_w, gate_b, w_gate, w_up, w_down, sw_gate, sw_up,
                 sw_down, assign, prof):
    CKMAX = P * max(prof)
    xT = np.ascontiguousarray(x.T.astype(np.float32))
    com = {
        "xTf": xT.reshape(ND, P, T),
        "xbf": _bf16(np.concatenate([x, np.zeros((CKMAX, D), np.float32)],
                                    axis=0)),
        "gwT": np.ascontiguousarray(
            gate_w.T.astype(np.float32).reshape(ND, P, E).transpose(1, 0, 2)),
        "gbias": gate_b.reshape(1, E).astype(np.float32),
        "ones1": np.ones((1, P), np.float32),
        "ut128": np.triu(np.ones((P, P), np.float32)),
        "iota128": np.arange(P, dtype=np.float32).reshape(P, 1),
        "initmeta": np.stack([np.arange(T, T + CKMAX, dtype=np.float32),
                              np.zeros(CKMAX, np.float32)], axis=1),
    }
    wc = np.zeros((P, NT * NT), np.float32)
    for jp in range(NT):
        for j in range(NT):
            if jp < j:
                wc[:, jp * NT + j] = 1.0
    com["wcum"] = wc
    ej = np.zeros((NT, NT * P), np.float32)
    for j in range(NT):
        ej[j, j * P:(j + 1) * P] = 1.0
    com["ejall"] = ej

    sgp = np.zeros((D, SIP), np.float32)
    sup = np.zeros((D, SIP), np.float32)
    sdp = np.zeros((SIP, D), np.float32)

    in_maps = []
    for c in range(NC):
        m = dict(com)
        oh = np.zeros((P, 4, E), np.float32)
        for k in range(4):
            oh[:, k, assign[c, k]] = 1.0
        m["onehot"] = oh
        for k in range(4):
            e = assign[c, k]
            m[f"wgu{k}"] = _bf16(rearrange(w_gate[e],
                                           "(dc p) (it i) -> it p dc i",
                                           p=P, i=P))
            m[f"wuu{k}"] = _bf16(rearrange(w_up[e],
                                           "(dc p) (it i) -> it p dc i",
                                           p=P, i=P))
            m[f"wdd{k}"] = _wd_flat(w_down[e], NI)
        sgp[:, :SI] = sw_gate[:, c * SI:(c + 1) * SI]
        sup[:, :SI] = sw_up[:, c * SI:(c + 1) * SI]
        sdp[:SI, :] = sw_down[c * SI:(c + 1) * SI, :]
        m["swg"] = _bf16(rearrange(sgp, "(dc p) (it i) -> it p dc i", p=P, i=P))
        m["swu"] = _bf16(rearrange(sup, "(dc p) (it i) -> it p dc i", p=P, i=P))
        m["swd"] = _wd_flat(sdp, NSI)
        in_maps.append(m)
    return in_maps


def _run(inputs, trace=False, tmpdir=None):
    x = np.asarray(inputs["x"], np.float32)
    gate_w = np.asarray(inputs["gate_w"], np.float32)
    gate_b = np.asarray(inputs["gate_b"], np.float32)
    counts = _routing_counts(x, gate_w, gate_b)
    assign, prof = _assignment(counts)
    if prof not in _program_cache:
        _program_cache[prof] = build_program(prof)
    nc = _program_cache[prof]
    in_maps = _prep_inputs(
        x, gate_w, gate_b,
        np.asarray(inputs["w_gate"], np.float32),
        np.asarray(inputs["w_up"], np.float32),
        np.asarray(inputs["w_down"], np.float32),
        np.asarray(inputs["sw_gate"], np.float32),
        np.asarray(inputs["sw_up"], np.float32),
        np.asarray(inputs["sw_down"], np.float32),
        assign, prof)
    res = run_bass_kernel_spmd(nc, in_maps, core_ids=list(range(NC)),
                               trace=trace, tmpdir=tmpdir)
    out = np.concatenate([np.asarray(res.results[c]["ysh"]) for c in range(NC)],
                         axis=0)
    return out.astype(np.float32), res


def kernel(**inputs):
    last = None
    for _attempt in range(3):
        try:
            out, _ = _run(inputs, trace=False)
            return out
        except Exception as e:  # transient NRT device errors: retry
            last = e
    raise last
